# revision 4
# baseline (speedup 1.0000x reference)
"""Trainium2 Bass kernel for nn_Attn (additive attention scores + softmax).

Math: with W split as [W1 | W2] (each [H, H]),
  scores[b, s] = v . (W1 @ hidden[b] + W2 @ enc[s, b] + bias)
               = (v @ W2) . enc[s, b]  +  const(b)
Softmax over s is shift-invariant, so const(b) drops out and
  out[b, 0, :] = softmax_s(enc[:, b, :] @ u2),   u2 = v @ W2  (a length-H vector).

So the kernel is a pure streaming dot-product over encoderOutputs plus a tiny
per-row softmax -- exactly memory-bound. enc and u2 are shipped as fp16
(input-quantization error on the softmax output is ~1e-3 relative, measured
against the f32 reference; the DVE accumulates in fp32 internally), which
halves HBM traffic and enables the DVE 2x packed perf mode.

Sharding: batch B=32 across 8 cores (4 batches per core), params replicated.
Per core 16 MiB is streamed once; each 128-row tile's dot with u2 is ONE DVE
scalar_tensor_tensor (out=(in0*1.0)*in1, accum_out=row-sum in fp32) -- a
single fused multiply+reduce pass.

Layout trick: per (core, b) the 4096 sequence positions are tiled as
s = 32*p + t (p = SBUF partition, t = score column). Input DMA then reads
contiguous multi-KiB runs per partition and the final [128, 32] probability
tile is one contiguous 16 KiB block in DRAM -- no transposes on the hot path.

Softmax cross-partition reductions use the PE (transpose for max,
matmul-with-ones for sum/broadcast); exp, psum->sbuf moves and the final
normalization run on the otherwise-idle Scalar engine (activation supports a
per-partition scale/bias AP and a fused free-dim accumulator).
"""

import numpy as np

_S, _H, _B = 4096, 512, 32
_NCORES, _BPC = 8, 4  # 8 cores x 4 batches per core
_P = 128  # SBUF partitions
_T = _S // _P  # 32 score columns per (core, b)
_TC = 16  # t-columns per DMA chunk (2 MiB fp16 per dma_start)
_NCH = _T // _TC

_cache = {}


def _build_program():
    import concourse.bacc as bacc
    import concourse.tile as tile
    from concourse import mybir

    f32 = mybir.dt.float32
    f16 = mybir.dt.float16
    nc = bacc.Bacc(
        "TRN2",
        target_bir_lowering=False,
        debug=False,
        enable_asserts=True,
        num_devices=_NCORES,
    )

    enc4 = nc.declare_dram_parameter("enc4", [_BPC, _P, _T, _H], f16, isOutput=False)
    u2r = nc.declare_dram_parameter("u2r", [_P, _H], f16, isOutput=False)
    ident = nc.declare_dram_parameter("ident", [_P, _P], f32, isOutput=False)
    out4 = nc.declare_dram_parameter("out4", [_BPC, _P, _T], f32, isOutput=True)

    with tile.TileContext(nc) as tc:
        with (
            tc.tile_pool(name="singles", bufs=1) as singles,
            tc.tile_pool(name="chunks", bufs=3) as chunks,
            tc.tile_pool(name="prod", bufs=2) as prodp,
            tc.tile_pool(name="scores", bufs=2) as scoresp,
            tc.tile_pool(name="soft", bufs=2) as soft,
            tc.tile_pool(name="small", bufs=4) as small,
            tc.tile_pool(name="psum", bufs=2, space="PSUM") as psum,
        ):
            u2t = singles.tile([_P, _H], f16)
            nc.sync.dma_start(out=u2t[:], in_=u2r[:, :])
            it = singles.tile([_P, _P], f32)
            nc.sync.dma_start(out=it[:], in_=ident[:, :])
            ones_col = singles.tile([_P, 1], f32)
            nc.vector.memset(ones_col[:], 1.0)
            ones_row = singles.tile([1, _P], f32)
            nc.vector.memset(ones_row[:], 1.0)
            negones_row = singles.tile([1, _P], f32)
            nc.vector.memset(negones_row[:], -1.0)

            for bb in range(_BPC):
                sc = scoresp.tile([_P, _T], f32, tag="sc")
                # Fine-grained leading DMAs for the very first batch so the DVE
                # starts ~2 us in instead of waiting for a full 2 MiB chunk.
                if bb == 0:
                    tcs = [2, 2, 4, 8, 16]
                else:
                    tcs = [_TC] * _NCH
                t0 = 0
                for tc_w in tcs:
                    et = chunks.tile([_P, _TC, _H], f16, tag="et")
                    nc.sync.dma_start(
                        out=et[:, :tc_w, :], in_=enc4[bb, :, t0 : t0 + tc_w, :]
                    )
                    for j in range(tc_w):
                        col = t0 + j
                        if tc_w == _TC and j % 4 == 3:
                            # Offload 1 in 4 dots of full-size chunks to the
                            # otherwise-idle GpSimd (multiply) + Scalar
                            # (fused accumulate) engines.
                            gprod = prodp.tile([_P, _H], f16, tag="gprod")
                            nc.gpsimd.tensor_tensor(
                                out=gprod[:],
                                in0=et[:, j, :],
                                in1=u2t[:],
                                op=mybir.AluOpType.mult,
                            )
                            gdump = prodp.tile([_P, 1], f16, tag="gdump")
                            nc.scalar.activation(
                                out=gdump[:].broadcast_to((_P, _H)),
                                in_=gprod[:],
                                func=mybir.ActivationFunctionType.Copy,
                                bias=0.0,
                                scale=1.0,
                                accum_out=sc[:, col : col + 1],
                            )
                        else:
                            prod = prodp.tile([_P, 1], f16, tag="prod")
                            nc.vector.scalar_tensor_tensor(
                                out=prod[:].broadcast_to((_P, _H)),
                                in0=et[:, j, :],
                                scalar=1.0,
                                in1=u2t[:],
                                op0=mybir.AluOpType.mult,
                                op1=mybir.AluOpType.mult,
                                accum_out=sc[:, col : col + 1],
                            )
                    t0 += tc_w

                # ---- softmax over the full [128, 32] score tile (one batch row)
                m1 = small.tile([_P, 1], f32, tag="m1")
                nc.vector.reduce_max(out=m1[:], in_=sc[:], axis=mybir.AxisListType.X)
                # cross-partition max: PE transpose -> [1, 128] -> free-dim max
                m1t = psum.tile([1, _P], f32, tag="m1t")
                nc.tensor.transpose(m1t[:], m1[:], it[:])
                mx = small.tile([1, 1], f32, tag="mx")
                nc.vector.reduce_max(out=mx[:], in_=m1t[:], axis=mybir.AxisListType.X)
                # broadcast -max to all partitions via PE: (-1s)^T @ mx
                negm_ps = psum.tile([_P, 1], f32, tag="negm_ps")
                nc.tensor.matmul(
                    negm_ps[:], lhsT=negones_row[:], rhs=mx[:], start=True, stop=True
                )
                negm = small.tile([_P, 1], f32, tag="negm")
                nc.scalar.copy(out=negm[:], in_=negm_ps[:])

                ex = soft.tile([_P, _T], f32, tag="ex")
                sumex = small.tile([_P, 1], f32, tag="sumex")
                nc.scalar.activation(
                    out=ex[:],
                    in_=sc[:],
                    func=mybir.ActivationFunctionType.Exp,
                    bias=negm[:],
                    scale=1.0,
                    accum_out=sumex[:],
                )
                # cross-partition sum: ones-weighted matmul -> Z [1,1]
                z_ps = psum.tile([1, 1], f32, tag="z_ps")
                nc.tensor.matmul(
                    z_ps[:], lhsT=sumex[:], rhs=ones_col[:], start=True, stop=True
                )
                rz = small.tile([1, 1], f32, tag="rz")
                nc.vector.reciprocal(out=rz[:], in_=z_ps[:])
                # broadcast 1/Z to all partitions
                rzb_ps = psum.tile([_P, 1], f32, tag="rzb_ps")
                nc.tensor.matmul(
                    rzb_ps[:], lhsT=ones_row[:], rhs=rz[:], start=True, stop=True
                )
                rzb = small.tile([_P, 1], f32, tag="rzb")
                nc.scalar.copy(out=rzb[:], in_=rzb_ps[:])

                # normalize on the Scalar engine: pb = ex * (1/Z) per partition
                pb = soft.tile([_P, _T], f32, tag="pb")
                nc.scalar.activation(
                    out=pb[:],
                    in_=ex[:],
                    func=mybir.ActivationFunctionType.Copy,
                    bias=0.0,
                    scale=rzb[:],
                )
                nc.scalar.dma_start(out=out4[bb, :, :], in_=pb[:])

    nc.compile()
    return nc


def _get_nc():
    if "nc" not in _cache:
        _cache["nc"] = _build_program()
    return _cache["nc"]


def _prep_in_maps(encoderOutputs, W, v):
    enc = np.asarray(encoderOutputs, dtype=np.float32)
    W = np.asarray(W, dtype=np.float32)
    v = np.asarray(v, dtype=np.float32)
    u2 = (v.astype(np.float64) @ W[:, _H:].astype(np.float64)).astype(np.float16)
    u2r = np.ascontiguousarray(np.broadcast_to(u2, (_P, _H)))
    ident = np.eye(_P, dtype=np.float32)
    in_maps = []
    for c in range(_NCORES):
        blk = np.ascontiguousarray(
            enc[:, c * _BPC : (c + 1) * _BPC, :].transpose(1, 0, 2)
        ).astype(np.float16)  # [BPC, S, H], b-major
        in_maps.append(
            {"enc4": blk.reshape(_BPC, _P, _T, _H), "u2r": u2r, "ident": ident}
        )
    return in_maps


def run_spmd(inputs, trace=False, **kwargs):
    """Run the SPMD kernel across 8 cores. Returns BassKernelResults."""
    from concourse.bass_utils import run_bass_kernel_spmd

    nc = _get_nc()
    in_maps = _prep_in_maps(inputs["encoderOutputs"], inputs["W"], inputs["v"])
    return run_bass_kernel_spmd(
        nc, in_maps, list(range(_NCORES)), trace=trace, **kwargs
    )


def _assemble(results):
    outs = [np.asarray(r["out4"], dtype=np.float32).reshape(_BPC, _S) for r in results]
    return np.concatenate(outs, axis=0)[:, None, :]


def kernel(hidden, encoderOutputs, W, b, v):
    res = run_spmd({"encoderOutputs": encoderOutputs, "W": W, "v": v})
    return _assemble(res.results)


# revision 6
# speedup vs baseline: 1.0614x; 1.0614x over previous
"""Trainium2 Bass kernel for nn_Attn (additive attention scores + softmax).

Math: with W split as [W1 | W2] (each [H, H]),
  scores[b, s] = v . (W1 @ hidden[b] + W2 @ enc[s, b] + bias)
               = (v @ W2) . enc[s, b]  +  const(b)
Softmax over s is shift-invariant, so const(b) drops out and
  out[b, 0, :] = softmax_s(enc[:, b, :] @ u2),   u2 = v @ W2  (a length-H vector).

So the kernel is a pure streaming dot-product over encoderOutputs plus a tiny
per-row softmax -- exactly memory-bound. enc and u2 are shipped as fp16
(input-quantization error on the softmax output is ~1e-3 relative, measured
against the f32 reference; the DVE accumulates in fp32 internally), which
halves HBM traffic and enables the DVE 2x packed perf mode.

Sharding: batch B=32 across 8 cores (4 batches per core), params replicated.
Per core 16 MiB is streamed once; each 128-row tile's dot with u2 is ONE DVE
scalar_tensor_tensor (out=(in0*1.0)*in1, accum_out=row-sum in fp32) -- a
single fused multiply+reduce pass.

Layout trick: per (core, b) the 4096 sequence positions are tiled as
s = 32*p + t (p = SBUF partition, t = score column). Input DMA then reads
contiguous multi-KiB runs per partition and the final [128, 32] probability
tile is one contiguous 16 KiB block in DRAM -- no transposes on the hot path.

Softmax cross-partition reductions use the PE (transpose for max,
matmul-with-ones for sum/broadcast); exp, psum->sbuf moves and the final
normalization run on the otherwise-idle Scalar engine (activation supports a
per-partition scale/bias AP and a fused free-dim accumulator).
"""

import numpy as np

_S, _H, _B = 4096, 512, 32
_NCORES, _BPC = 8, 4  # 8 cores x 4 batches per core
_P = 128  # SBUF partitions
_T = _S // _P  # 32 score columns per (core, b)
_TC = 32  # t-columns per DMA chunk (4 MiB fp16 per dma_start)
_NCH = _T // _TC

_cache = {}


def _build_program():
    import concourse.bacc as bacc
    import concourse.tile as tile
    from concourse import mybir

    f32 = mybir.dt.float32
    f16 = mybir.dt.float16
    nc = bacc.Bacc(
        "TRN2",
        target_bir_lowering=False,
        debug=False,
        enable_asserts=True,
        num_devices=_NCORES,
    )

    enc4 = nc.declare_dram_parameter("enc4", [_BPC, _P, _T, _H], f16, isOutput=False)
    u2r = nc.declare_dram_parameter("u2r", [_P, _H], f16, isOutput=False)
    ident = nc.declare_dram_parameter("ident", [_P, _P], f32, isOutput=False)
    out4 = nc.declare_dram_parameter("out4", [_BPC, _P, _T], f32, isOutput=True)

    with tile.TileContext(nc) as tc:
        with (
            tc.tile_pool(name="singles", bufs=1) as singles,
            tc.tile_pool(name="chunks", bufs=3) as chunks,
            tc.tile_pool(name="prod", bufs=2) as prodp,
            tc.tile_pool(name="scores", bufs=2) as scoresp,
            tc.tile_pool(name="soft", bufs=2) as soft,
            tc.tile_pool(name="small", bufs=4) as small,
            tc.tile_pool(name="psum", bufs=2, space="PSUM") as psum,
        ):
            u2t = singles.tile([_P, _H], f16)
            nc.sync.dma_start(out=u2t[:], in_=u2r[:, :])
            it = singles.tile([_P, _P], f32)
            nc.sync.dma_start(out=it[:], in_=ident[:, :])
            ones_col = singles.tile([_P, 1], f32)
            nc.vector.memset(ones_col[:], 1.0)
            ones_row = singles.tile([1, _P], f32)
            nc.vector.memset(ones_row[:], 1.0)
            negones_row = singles.tile([1, _P], f32)
            nc.vector.memset(negones_row[:], -1.0)

            for bb in range(_BPC):
                sc = scoresp.tile([_P, _T], f32, tag="sc")
                # Fine-grained leading DMAs for the very first batch so the DVE
                # starts ~2 us in instead of waiting for a full-size chunk.
                if bb == 0:
                    tcs = [2, 2, 4, 8, 16]
                else:
                    tcs = [_TC] * _NCH
                t0 = 0
                for tc_w in tcs:
                    et = chunks.tile([_P, _TC, _H], f16, tag="et")
                    nc.sync.dma_start(
                        out=et[:, :tc_w, :], in_=enc4[bb, :, t0 : t0 + tc_w, :]
                    )
                    for j in range(tc_w):
                        col = t0 + j
                        prod = prodp.tile([_P, 1], f16, tag="prod")
                        nc.vector.scalar_tensor_tensor(
                            out=prod[:].broadcast_to((_P, _H)),
                            in0=et[:, j, :],
                            scalar=1.0,
                            in1=u2t[:],
                            op0=mybir.AluOpType.mult,
                            op1=mybir.AluOpType.mult,
                            accum_out=sc[:, col : col + 1],
                        )
                    t0 += tc_w

                # ---- softmax over the full [128, 32] score tile (one batch row)
                m1 = small.tile([_P, 1], f32, tag="m1")
                nc.vector.reduce_max(out=m1[:], in_=sc[:], axis=mybir.AxisListType.X)
                # cross-partition max: PE transpose -> [1, 128] -> free-dim max
                m1t = psum.tile([1, _P], f32, tag="m1t")
                nc.tensor.transpose(m1t[:], m1[:], it[:])
                mx = small.tile([1, 1], f32, tag="mx")
                nc.vector.reduce_max(out=mx[:], in_=m1t[:], axis=mybir.AxisListType.X)
                # broadcast -max to all partitions via PE: (-1s)^T @ mx
                negm_ps = psum.tile([_P, 1], f32, tag="negm_ps")
                nc.tensor.matmul(
                    negm_ps[:], lhsT=negones_row[:], rhs=mx[:], start=True, stop=True
                )
                negm = small.tile([_P, 1], f32, tag="negm")
                nc.scalar.copy(out=negm[:], in_=negm_ps[:])

                ex = soft.tile([_P, _T], f32, tag="ex")
                sumex = small.tile([_P, 1], f32, tag="sumex")
                nc.scalar.activation(
                    out=ex[:],
                    in_=sc[:],
                    func=mybir.ActivationFunctionType.Exp,
                    bias=negm[:],
                    scale=1.0,
                    accum_out=sumex[:],
                )
                # cross-partition sum: ones-weighted matmul -> Z [1,1]
                z_ps = psum.tile([1, 1], f32, tag="z_ps")
                nc.tensor.matmul(
                    z_ps[:], lhsT=sumex[:], rhs=ones_col[:], start=True, stop=True
                )
                rz = small.tile([1, 1], f32, tag="rz")
                nc.vector.reciprocal(out=rz[:], in_=z_ps[:])
                # broadcast 1/Z to all partitions
                rzb_ps = psum.tile([_P, 1], f32, tag="rzb_ps")
                nc.tensor.matmul(
                    rzb_ps[:], lhsT=ones_row[:], rhs=rz[:], start=True, stop=True
                )
                rzb = small.tile([_P, 1], f32, tag="rzb")
                nc.scalar.copy(out=rzb[:], in_=rzb_ps[:])

                # normalize on the Scalar engine: pb = ex * (1/Z) per partition
                pb = soft.tile([_P, _T], f32, tag="pb")
                nc.scalar.activation(
                    out=pb[:],
                    in_=ex[:],
                    func=mybir.ActivationFunctionType.Copy,
                    bias=0.0,
                    scale=rzb[:],
                )
                nc.scalar.dma_start(out=out4[bb, :, :], in_=pb[:])

    nc.compile()
    return nc


def _get_nc():
    if "nc" not in _cache:
        _cache["nc"] = _build_program()
    return _cache["nc"]


def _prep_in_maps(encoderOutputs, W, v):
    enc = np.asarray(encoderOutputs, dtype=np.float32)
    W = np.asarray(W, dtype=np.float32)
    v = np.asarray(v, dtype=np.float32)
    u2 = (v.astype(np.float64) @ W[:, _H:].astype(np.float64)).astype(np.float16)
    u2r = np.ascontiguousarray(np.broadcast_to(u2, (_P, _H)))
    ident = np.eye(_P, dtype=np.float32)
    in_maps = []
    for c in range(_NCORES):
        blk = np.ascontiguousarray(
            enc[:, c * _BPC : (c + 1) * _BPC, :].transpose(1, 0, 2)
        ).astype(np.float16)  # [BPC, S, H], b-major
        in_maps.append(
            {"enc4": blk.reshape(_BPC, _P, _T, _H), "u2r": u2r, "ident": ident}
        )
    return in_maps


def run_spmd(inputs, trace=False, **kwargs):
    """Run the SPMD kernel across 8 cores. Returns BassKernelResults."""
    from concourse.bass_utils import run_bass_kernel_spmd

    nc = _get_nc()
    in_maps = _prep_in_maps(inputs["encoderOutputs"], inputs["W"], inputs["v"])
    return run_bass_kernel_spmd(
        nc, in_maps, list(range(_NCORES)), trace=trace, **kwargs
    )


def _assemble(results):
    outs = [np.asarray(r["out4"], dtype=np.float32).reshape(_BPC, _S) for r in results]
    return np.concatenate(outs, axis=0)[:, None, :]


def kernel(hidden, encoderOutputs, W, b, v):
    res = run_spmd({"encoderOutputs": encoderOutputs, "W": W, "v": v})
    return _assemble(res.results)
